# revision 24
# baseline (speedup 1.0000x reference)
"""Trainium2 Bass kernel for nn_NLL_87333864997268 (GLMM logistic NLL with
Gauss-Hermite quadrature over a random intercept).

Math
----
With y in {0,1}, f the logit, c_k = sqrt(2*sig2b)*x_k (GH nodes):

    T[k,q] = sum_{i in group q} [ softplus(f_i + c_k) - y_i*(f_i + c_k) ]
           = SP_k[q] - YF[q] - c_k*SY[q]        (all three are segment sums)
    loss = -sum_q log( sum_k w_k/sqrt(pi) * exp(-T[k,q]) )

Strategy
--------
Host: stable sort by group id; pad each group to fixed-width W=52 "pieces"
(ceil(size/52) pieces per group, ~+37% slots); pack pieces into 1024
partition rows (8 cores x 128 partitions), never splitting a group across
partitions. All segment sums then become dense fixed-stride reductions on
device; host supplies two {0,1} masks: m (piece j continues into j+1) and
z (piece j is the first piece of its group).

Device (per core): 3 dense reduces for y and y*f, then per quadrature node
softplus (exp+ln, no Softplus table in this build) and a dense piece-sum
reduce; combine pieces via the m-mask (twice, supporting groups up to 3
pieces); stabilized log-sum-exp over the 5 nodes; z-masked total -> one
scalar per core. Host sums the 8 partials. Pad slots use f=-1e4, y=0 so
softplus(f+c)=0 exactly and they contribute nothing.
"""

import numpy as np

import concourse.bacc as bacc
import concourse.bass as bass
import concourse.mybir as mybir
import concourse.tile as tile
from concourse.bass_utils import run_bass_kernel_spmd

# problem constants (hardcoded per spec)
N = 4_194_304
Q = 100_000
NCORES = 8
K = 5            # Gauss-Hermite nodes
PT = 128         # partitions per core
NPART = NCORES * PT

WB = 52          # piece width (slots per piece)
NP = 108         # pieces per partition (static capacity)
FT = NP * WB     # slots per partition = 5616
FPAD = -10000.0  # pad logit: softplus(fpad+c)=0, y=0

_XK, _WK = np.polynomial.hermite.hermgauss(K)

F32 = mybir.dt.float32
BF16 = mybir.dt.bfloat16


def build_nc(dbg=False):
    """Build + compile the single-core SPMD Bass program."""
    nc = bacc.Bacc("TRN2", target_bir_lowering=False, debug=False)

    ys_d = nc.dram_tensor("ys", [PT, FT], BF16, kind="ExternalInput")
    fs_d = nc.dram_tensor("fs", [PT, FT], BF16, kind="ExternalInput")
    m5_d = nc.dram_tensor("m5", [PT, NP * K], F32, kind="ExternalInput")
    z_d = nc.dram_tensor("z", [PT, NP], F32, kind="ExternalInput")
    cbias_d = nc.dram_tensor("cbias", [PT, K], F32, kind="ExternalInput")
    escale_d = nc.dram_tensor("escale", [PT, K], F32, kind="ExternalInput")
    wtile_d = nc.dram_tensor("wtile", [PT, NP * K], F32, kind="ExternalInput")
    loss_d = nc.dram_tensor("loss", [1, 1], F32, kind="ExternalOutput")
    dbg_t = {}
    if dbg:
        dbg_t["dbg_T"] = nc.dram_tensor("dbg_T", [PT, NP * K], F32, kind="ExternalOutput")
        dbg_t["dbg_P"] = nc.dram_tensor("dbg_P", [PT, NP * K], F32, kind="ExternalOutput")
        dbg_t["dbg_dq"] = nc.dram_tensor("dbg_dq", [PT, NP], F32, kind="ExternalOutput")

    ADD = mybir.AluOpType.add
    SUB = mybir.AluOpType.subtract
    MULT = mybir.AluOpType.mult
    MIN = mybir.AluOpType.min
    ACT = mybir.ActivationFunctionType
    AX = mybir.AxisListType.X

    with tile.TileContext(nc) as tc:
        with (
            tc.tile_pool(name="big", bufs=1) as big,
            tc.tile_pool(name="tmp", bufs=2) as tmp,
            tc.tile_pool(name="small", bufs=1) as small,
            tc.tile_pool(name="psum", bufs=1, space="PSUM") as psum,
        ):
            yt = big.tile([PT, FT], BF16, tag="yt")
            ft = big.tile([PT, FT], BF16, tag="ft")

            cb = small.tile([PT, K], F32, tag="cb")
            es = small.tile([PT, K], F32, tag="es")
            m5 = small.tile([PT, NP * K], F32, tag="m5")
            zt = small.tile([PT, NP], F32, tag="zt")
            wt = small.tile([PT, NP * K], F32, tag="wt")
            nc.sync.dma_start(out=cb[:], in_=cbias_d[:])
            nc.sync.dma_start(out=es[:], in_=escale_d[:])

            # ---- big elementwise + piece-sum stage, in CH chunks for pipelining.
            # Piece sums via pairwise-halving tree (bf16 tensor_tensor hits the
            # DVE 2x packed mode; tensor_reduce never does) + final 13-reduce.
            CH = 2
            FC = FT // CH          # slots per chunk (2808)
            NC_ = NP // CH         # pieces per chunk (54)
            assert FC * CH == FT and NC_ * CH == NP

            sy = small.tile([PT, NP], BF16, tag="sy")
            syf = small.tile([PT, NP], BF16, tag="syf")
            # T has one extra zero piece so the combine can read a shifted view
            T = small.tile([PT, (NP + 1) * K], F32, tag="T")
            nc.vector.memset(T[:, NP * K :], 0.0)
            T3 = T[:, : NP * K].rearrange("p (n k) -> p n k", k=K)
            cvu = small.tile([PT, NP * K], F32, tag="cvu")
            cvu3 = cvu[:].rearrange("p (n k) -> p n k", k=K)

            def tree_sum(src_ap, out_ap, npieces, l1_engine=nc.vector):
                """src [PT, npieces*WB] bf16 -> out [PT, npieces] bf16 piece sums."""
                s3 = src_ap.rearrange("p (n w) -> p n w", w=WB)
                h1 = tmp.tile([PT, npieces * 26], BF16, tag="h1")
                h13 = tmp.tile([PT, npieces * 13], BF16, tag="h13")
                h1v = h1[:].rearrange("p (n w) -> p n w", w=26)
                h13v = h13[:].rearrange("p (n w) -> p n w", w=13)
                l1_engine.tensor_tensor(out=h1v, in0=s3[:, :, 0:26], in1=s3[:, :, 26:52], op=ADD)
                nc.vector.tensor_tensor(out=h13v, in0=h1v[:, :, 0:13], in1=h1v[:, :, 13:26], op=ADD)
                nc.vector.tensor_reduce(out=out_ap, in_=h13v, axis=AX, op=ADD)

            with nc.allow_low_precision("piece sums are <=52 adds; bf16 keeps DVE 2x mode"):
                # phase 1: input DMAs + e^f per chunk (exp table loaded once)
                efs = []
                for c in range(CH):
                    fsl = slice(c * FC, (c + 1) * FC)
                    nc.sync.dma_start(out=ft[:, fsl], in_=fs_d[:, fsl])
                    nc.sync.dma_start(out=yt[:, fsl], in_=ys_d[:, fsl])
                    ef = big.tile([PT, FC], BF16, tag=f"ef{c}")
                    nc.scalar.activation(out=ef[:], in_=ft[:, fsl], func=ACT.Exp)
                    efs.append(ef)

                # phase 2: softplus(f+c_k) = ln(e^{c_k} * e^f + 1) + piece-sum trees
                # (emitted before the y-path so the in-order DVE queue isn't
                # blocked waiting on gpsimd's first levels)
                spns = {}
                for c in range(CH):
                    for k in range(K):
                        sp = tmp.tile([PT, FC], BF16, tag="sp")
                        nc.scalar.activation(
                            out=sp[:], in_=efs[c][:], func=ACT.Ln, bias=1.0,
                            scale=es[:, k : k + 1],
                        )
                        spn = tmp.tile([PT, NC_], BF16, tag=f"spn{c}_{k}")
                        tree_sum(sp[:], spn[:], NC_)
                        spns[(c, k)] = spn

                # phase 3: y / y*f piece sums (first tree level on gpsimd) + T
                for c in range(CH):
                    fsl = slice(c * FC, (c + 1) * FC)
                    nsl = slice(c * NC_, (c + 1) * NC_)
                    yf = tmp.tile([PT, FC], BF16, tag="yf")
                    nc.gpsimd.tensor_tensor(out=yf[:], in0=yt[:, fsl], in1=ft[:, fsl], op=MULT)
                    tree_sum(yt[:, fsl], sy[:, nsl], NC_, l1_engine=nc.gpsimd)
                    tree_sum(yf[:], syf[:, nsl], NC_, l1_engine=nc.gpsimd)
                    for k in range(K):
                        nc.vector.scalar_tensor_tensor(
                            out=cvu3[:, nsl, k], in0=sy[:, nsl], scalar=cb[:, k : k + 1],
                            in1=syf[:, nsl], op0=MULT, op1=ADD,
                        )
                    for k in range(K):
                        # negated T pieces: T3_k = (c_k*sy + syf) - spn
                        nc.vector.tensor_tensor(
                            out=T3[:, nsl, k], in0=cvu3[:, nsl, k], in1=spns[(c, k)][:], op=SUB
                        )

            # mask/weight inputs are only needed at the tail
            nc.sync.dma_start(out=m5[:], in_=m5_d[:])
            nc.sync.dma_start(out=zt[:], in_=z_d[:])
            nc.sync.dma_start(out=wt[:], in_=wtile_d[:])
            # NOTE: T currently holds  c_k*sy - spn + syf  = -(T_k). We work with
            # negT below: ksum = sum_k w_k * exp(negT - negTmax), loss_q = -negTmax - ln ksum.
            negT = T[:, : NP * K]

            # piece combine: PC_j = negT_j + m_j*(negT_{j+1} + m_{j+1}*negT_{j+2})
            # (supports groups spanning up to 3 pieces; host asserts that)
            pc1 = small.tile([PT, (NP + 1) * K], F32, tag="pc1")
            nc.vector.memset(pc1[:, NP * K :], 0.0)
            t2 = tmp.tile([PT, NP * K], F32, tag="t2")
            nc.vector.tensor_tensor(out=t2[:], in0=T[:, K :], in1=m5[:], op=MULT)
            nc.vector.tensor_tensor(out=pc1[:, : NP * K], in0=negT, in1=t2[:], op=ADD)
            pc = small.tile([PT, NP * K], F32, tag="pc")
            t3 = tmp.tile([PT, NP * K], F32, tag="t3")
            nc.vector.tensor_tensor(out=t3[:], in0=pc1[:, K :], in1=m5[:], op=MULT)
            nc.vector.tensor_tensor(out=pc[:], in0=negT, in1=t3[:], op=ADD)

            # stabilized LSE over k on combined pieces
            nmax = small.tile([PT, NP], F32, tag="nmax")
            pc3 = pc[:].rearrange("p (n k) -> p n k", k=K)
            nc.vector.tensor_reduce(out=nmax[:], in_=pc3, axis=AX, op=mybir.AluOpType.max)
            nmax_b = nmax[:].unsqueeze(2).broadcast_to((PT, NP, K))
            nc.vector.tensor_tensor(out=pc3, in0=pc3, in1=nmax_b, op=SUB)
            ex = small.tile([PT, NP * K], F32, tag="ex")
            nc.scalar.activation(out=ex[:], in_=pc[:], func=ACT.Exp)
            wm = small.tile([PT, NP * K], F32, tag="wm")
            nc.vector.tensor_tensor(out=wm[:], in0=ex[:], in1=wt[:], op=MULT)
            ks = small.tile([PT, NP], F32, tag="ks")
            nc.vector.tensor_reduce(
                out=ks[:], in_=wm[:].rearrange("p (n k) -> p n k", k=K), axis=AX, op=ADD
            )
            lk = small.tile([PT, NP], F32, tag="lk")
            nc.scalar.activation(out=lk[:], in_=ks[:], func=ACT.Ln)
            # loss_q = -log ksum_full = -(nmax + lk)  => accumulate (nmax+lk), negate at end
            dq = small.tile([PT, NP], F32, tag="dq")
            nc.vector.tensor_tensor(out=dq[:], in0=nmax[:], in1=lk[:], op=ADD)
            dqz = small.tile([PT, NP], F32, tag="dqz")
            nc.vector.tensor_tensor(out=dqz[:], in0=dq[:], in1=zt[:], op=MULT)
            if dbg:
                nc.sync.dma_start(out=dbg_t["dbg_T"][:], in_=negT[:])
                nc.sync.dma_start(out=dbg_t["dbg_P"][:], in_=pc[:])
                nc.sync.dma_start(out=dbg_t["dbg_dq"][:], in_=dqz[:])
            rs = small.tile([PT, 1], F32, tag="rs")
            nc.vector.tensor_reduce(out=rs[:], in_=dqz[:], axis=AX, op=ADD)
            negones = small.tile([PT, 1], F32, tag="negones")
            nc.vector.memset(negones[:], -1.0)
            tot_p = psum.tile([1, 1], F32)
            nc.tensor.matmul(out=tot_p[:], lhsT=rs[:], rhs=negones[:], start=True, stop=True)
            tot = small.tile([1, 1], F32, tag="tot")
            nc.vector.tensor_copy(out=tot[:], in_=tot_p[:])
            nc.sync.dma_start(out=loss_d[:], in_=tot[:])

    nc.compile()
    return nc


_NC_CACHE = {}


def get_nc(dbg=False):
    if dbg not in _NC_CACHE:
        _NC_CACHE[dbg] = build_nc(dbg)
    return _NC_CACHE[dbg]


def host_prep(y_true, y_pred, Z_idx, sig2b):
    """Sort by group; pack groups into fixed-width pieces across 1024 partitions."""
    y = np.asarray(y_true, dtype=np.float32).reshape(-1)
    f = np.asarray(y_pred, dtype=np.float32).reshape(-1)
    idx = np.asarray(Z_idx).astype(np.int32)
    n = y.shape[0]
    assert n == N

    perm = np.argsort(idx, kind="stable")
    sb = idx[perm]
    ys = y[perm]
    fs = f[perm]

    s = np.bincount(sb, minlength=Q).astype(np.int64)          # group sizes
    bin_start = np.concatenate(([0], np.cumsum(s)[:-1]))
    pcs = (s + WB - 1) // WB                                   # pieces per group
    piece_off = np.concatenate(([0], np.cumsum(pcs)[:-1]))
    total_pieces = int(pcs.sum())
    npt = -(-total_pieces // NPART)                            # target pieces/partition
    assert npt + int(pcs.max()) - 1 <= NP, (npt, int(pcs.max()))
    assert int(pcs.max()) <= 3, int(pcs.max())                 # combine depth

    nz = s > 0
    pid = np.zeros(Q, np.int64)
    pid[nz] = piece_off[nz] // npt                             # partition of each group
    assert pid.max() < NPART

    # local piece base per group: piece_off - first piece_off in its partition
    first_bin = np.searchsorted(pid[nz], np.arange(NPART), side="left")
    po_nz = piece_off[nz]
    part_first = np.zeros(NPART, np.int64)
    valid = first_bin < po_nz.shape[0]
    part_first[valid] = po_nz[np.minimum(first_bin, po_nz.shape[0] - 1)][valid]
    lpi = np.zeros(Q, np.int64)
    lpi[nz] = piece_off[nz] - part_first[pid[nz]]
    assert (lpi[nz] + pcs[nz]).max() <= NP

    # per-element slot
    b = sb.astype(np.int64)
    r = np.arange(n, dtype=np.int64) - bin_start[b]            # rank within group
    slot = pid[b] * FT + (lpi[b] + r // WB) * WB + (r % WB)

    Y = np.zeros(NPART * FT, np.float32)
    F = np.full(NPART * FT, FPAD, np.float32)
    Y[slot] = ys
    F[slot] = fs

    # masks
    mflat = np.zeros(NPART * NP, np.float32)
    for extra in (1, 2):
        sel = pcs > extra
        mflat[(pid[sel] * NP + lpi[sel] + (extra - 1)).astype(np.int64)] = 1.0
    zflat = np.zeros(NPART * NP, np.float32)
    zflat[(pid[nz] * NP + lpi[nz]).astype(np.int64)] = 1.0

    sig = float(np.asarray(sig2b).reshape(-1)[0])
    ck = (np.sqrt(2.0 * sig) * _XK).astype(np.float32)
    wk = (_WK / np.sqrt(np.pi)).astype(np.float32)
    cbias = np.tile(ck[None, :], (PT, 1)).astype(np.float32)
    escale = np.tile(np.exp(ck.astype(np.float64))[None, :], (PT, 1)).astype(np.float32)
    wtile = np.tile(wk[None, :], (PT, NP)).astype(np.float32)

    bf16 = mybir.dt.np(BF16)
    Y = Y.reshape(NPART, FT).astype(bf16)
    F = F.reshape(NPART, FT).astype(bf16)
    m5 = np.repeat(mflat.reshape(NPART, NP), K, axis=1)        # [NPART, NP*K]
    z2 = zflat.reshape(NPART, NP)

    in_maps = []
    for c in range(NCORES):
        sl = slice(c * PT, (c + 1) * PT)
        in_maps.append(
            {
                "ys": Y[sl],
                "fs": F[sl],
                "m5": m5[sl],
                "z": z2[sl],
                "cbias": cbias,
                "escale": escale,
                "wtile": wtile,
            }
        )
    return in_maps


def finish(results):
    total = sum(float(results[c]["loss"][0, 0]) for c in range(NCORES))
    return np.float32(total)


def kernel(y_true, y_pred, Z_idx, sig2b):
    nc = get_nc()
    in_maps = host_prep(y_true, y_pred, Z_idx, sig2b)
    res = run_bass_kernel_spmd(nc, in_maps, list(range(NCORES)))
    return finish(res.results)


# revision 28
# speedup vs baseline: 1.0138x; 1.0138x over previous
"""Trainium2 Bass kernel for nn_NLL_87333864997268 (GLMM logistic NLL with
Gauss-Hermite quadrature over a random intercept).

Math
----
With y in {0,1}, f the logit, c_k = sqrt(2*sig2b)*x_k (GH nodes):

    T[k,q] = sum_{i in group q} [ softplus(f_i + c_k) - y_i*(f_i + c_k) ]
           = SP_k[q] - YF[q] - c_k*SY[q]        (all three are segment sums)
    loss = -sum_q log( sum_k w_k/sqrt(pi) * exp(-T[k,q]) )

Strategy
--------
Host: stable sort by group id; pad each group to fixed-width W=52 "pieces"
(ceil(size/52) pieces per group, ~+37% slots); pack pieces into 1024
partition rows (8 cores x 128 partitions), never splitting a group across
partitions. All segment sums then become dense fixed-stride reductions on
device; host supplies two {0,1} masks: m (piece j continues into j+1) and
z (piece j is the first piece of its group).

Device (per core): 3 dense reduces for y and y*f, then per quadrature node
softplus (exp+ln, no Softplus table in this build) and a dense piece-sum
reduce; combine pieces via the m-mask (twice, supporting groups up to 3
pieces); stabilized log-sum-exp over the 5 nodes; z-masked total -> one
scalar per core. Host sums the 8 partials. Pad slots use f=-1e4, y=0 so
softplus(f+c)=0 exactly and they contribute nothing.
"""

import numpy as np

import concourse.bacc as bacc
import concourse.bass as bass
import concourse.mybir as mybir
import concourse.tile as tile
from concourse.bass_utils import run_bass_kernel_spmd

# problem constants (hardcoded per spec)
N = 4_194_304
Q = 100_000
NCORES = 8
K = 5            # Gauss-Hermite nodes
PT = 128         # partitions per core
NPART = NCORES * PT

WB = 52          # piece width (slots per piece)
NP = 108         # pieces per partition (static capacity)
FT = NP * WB     # slots per partition = 5616
FPAD = -10000.0  # pad logit: softplus(fpad+c)=0, y=0

_XK, _WK = np.polynomial.hermite.hermgauss(K)

F32 = mybir.dt.float32
BF16 = mybir.dt.bfloat16


def build_nc(dbg=False):
    """Build + compile the single-core SPMD Bass program."""
    nc = bacc.Bacc("TRN2", target_bir_lowering=False, debug=False)

    ys_d = nc.dram_tensor("ys", [PT, FT], BF16, kind="ExternalInput")
    fs_d = nc.dram_tensor("fs", [PT, FT], BF16, kind="ExternalInput")
    m5_d = nc.dram_tensor("m5", [PT, NP * K], F32, kind="ExternalInput")
    z_d = nc.dram_tensor("z", [PT, NP], F32, kind="ExternalInput")
    cbias_d = nc.dram_tensor("cbias", [PT, K], F32, kind="ExternalInput")
    escale_d = nc.dram_tensor("escale", [PT, K], F32, kind="ExternalInput")
    wtile_d = nc.dram_tensor("wtile", [PT, NP * K], F32, kind="ExternalInput")
    loss_d = nc.dram_tensor("loss", [1, 1], F32, kind="ExternalOutput")
    dbg_t = {}
    if dbg:
        dbg_t["dbg_T"] = nc.dram_tensor("dbg_T", [PT, NP * K], F32, kind="ExternalOutput")
        dbg_t["dbg_P"] = nc.dram_tensor("dbg_P", [PT, NP * K], F32, kind="ExternalOutput")
        dbg_t["dbg_dq"] = nc.dram_tensor("dbg_dq", [PT, NP], F32, kind="ExternalOutput")

    ADD = mybir.AluOpType.add
    SUB = mybir.AluOpType.subtract
    MULT = mybir.AluOpType.mult
    MIN = mybir.AluOpType.min
    ACT = mybir.ActivationFunctionType
    AX = mybir.AxisListType.X

    with tile.TileContext(nc) as tc:
        with (
            tc.tile_pool(name="big", bufs=1) as big,
            tc.tile_pool(name="tmp", bufs=2) as tmp,
            tc.tile_pool(name="small", bufs=1) as small,
            tc.tile_pool(name="psum", bufs=1, space="PSUM") as psum,
        ):
            yt = big.tile([PT, FT], BF16, tag="yt")
            ft = big.tile([PT, FT], BF16, tag="ft")

            cb = small.tile([PT, K], F32, tag="cb")
            es = small.tile([PT, K], F32, tag="es")
            m5 = small.tile([PT, NP * K], F32, tag="m5")
            zt = small.tile([PT, NP], F32, tag="zt")
            wt = small.tile([PT, NP * K], F32, tag="wt")
            nc.sync.dma_start(out=cb[:], in_=cbias_d[:])
            nc.sync.dma_start(out=es[:], in_=escale_d[:])

            # ---- big elementwise + piece-sum stage, in CH chunks for pipelining.
            # Piece sums via pairwise-halving tree (bf16 tensor_tensor hits the
            # DVE 2x packed mode; tensor_reduce never does) + final 13-reduce.
            CH = 2
            FC = FT // CH          # slots per chunk (2808)
            NC_ = NP // CH         # pieces per chunk (54)
            assert FC * CH == FT and NC_ * CH == NP

            sy = small.tile([PT, NP], BF16, tag="sy")
            syf = small.tile([PT, NP], BF16, tag="syf")
            # T has one extra zero piece so the combine can read a shifted view
            T = small.tile([PT, (NP + 1) * K], F32, tag="T")
            nc.vector.memset(T[:, NP * K :], 0.0)
            T3 = T[:, : NP * K].rearrange("p (n k) -> p n k", k=K)
            cvu = small.tile([PT, NP * K], F32, tag="cvu")
            cvu3 = cvu[:].rearrange("p (n k) -> p n k", k=K)

            def tree_sum(src_ap, out_ap, npieces, l1_engine=nc.vector, tags=""):
                """src [PT, npieces*WB] bf16 -> out [PT, npieces] bf16 piece sums."""
                s3 = src_ap.rearrange("p (n w) -> p n w", w=WB)
                h1 = tmp.tile([PT, npieces * 26], BF16, tag="h1" + tags)
                h13 = tmp.tile([PT, npieces * 13], BF16, tag="h13" + tags)
                h1v = h1[:].rearrange("p (n w) -> p n w", w=26)
                h13v = h13[:].rearrange("p (n w) -> p n w", w=13)
                l1_engine.tensor_tensor(out=h1v, in0=s3[:, :, 0:26], in1=s3[:, :, 26:52], op=ADD)
                nc.vector.tensor_tensor(out=h13v, in0=h1v[:, :, 0:13], in1=h1v[:, :, 13:26], op=ADD)
                nc.vector.tensor_reduce(out=out_ap, in_=h13v, axis=AX, op=ADD)

            with nc.allow_low_precision("piece sums are <=52 adds; bf16 keeps DVE 2x mode"):
                # phase 1: input DMAs + e^f per chunk (exp table loaded once)
                efs = []
                exp_insts = []
                for c in range(CH):
                    fsl = slice(c * FC, (c + 1) * FC)
                    nc.sync.dma_start(out=ft[:, fsl], in_=fs_d[:, fsl])
                    nc.sync.dma_start(out=yt[:, fsl], in_=ys_d[:, fsl])
                    ef = big.tile([PT, FC], BF16, tag=f"ef{c}")
                    exp_insts.append(
                        nc.scalar.activation(out=ef[:], in_=ft[:, fsl], func=ACT.Exp)
                    )
                    efs.append(ef)

                # phase 2: softplus(f+c_k) = ln(e^{c_k} * e^f + 1) + piece-sum trees
                # (emitted before the y-path so the in-order DVE queue isn't
                # blocked waiting on gpsimd's first levels)
                spns = {}
                from concourse.tile import add_dep_helper
                for c in range(CH):
                    for k in range(K):
                        sp = tmp.tile([PT, FC], BF16, tag="sp")
                        ln_inst = nc.scalar.activation(
                            out=sp[:], in_=efs[c][:], func=ACT.Ln, bias=1.0,
                            scale=es[:, k : k + 1],
                        )
                        if c == 0 and k == 0:
                            # keep both exp passes under one exp-table period:
                            # the first ln must not slot between them
                            add_dep_helper(
                                ln_inst.ins, exp_insts[-1].ins, sync=False,
                                reason="act table grouping",
                            )
                        spn = tmp.tile([PT, NC_], BF16, tag=f"spn{c}_{k}")
                        tree_sum(sp[:], spn[:], NC_)
                        spns[(c, k)] = spn

                # phase 3: y / y*f piece sums (first tree level on gpsimd) + T
                for c in range(CH):
                    fsl = slice(c * FC, (c + 1) * FC)
                    nsl = slice(c * NC_, (c + 1) * NC_)
                    yf = tmp.tile([PT, FC], BF16, tag="yf")
                    nc.gpsimd.tensor_tensor(out=yf[:], in0=yt[:, fsl], in1=ft[:, fsl], op=MULT)
                    tree_sum(yt[:, fsl], sy[:, nsl], NC_, l1_engine=nc.gpsimd, tags="y")
                    tree_sum(yf[:], syf[:, nsl], NC_, l1_engine=nc.gpsimd, tags="y")
                    for k in range(K):
                        nc.vector.scalar_tensor_tensor(
                            out=cvu3[:, nsl, k], in0=sy[:, nsl], scalar=cb[:, k : k + 1],
                            in1=syf[:, nsl], op0=MULT, op1=ADD,
                        )
                    for k in range(K):
                        # negated T pieces: T3_k = (c_k*sy + syf) - spn
                        nc.vector.tensor_tensor(
                            out=T3[:, nsl, k], in0=cvu3[:, nsl, k], in1=spns[(c, k)][:], op=SUB
                        )

            # mask/weight inputs are only needed at the tail
            nc.sync.dma_start(out=m5[:], in_=m5_d[:])
            nc.sync.dma_start(out=zt[:], in_=z_d[:])
            nc.sync.dma_start(out=wt[:], in_=wtile_d[:])
            # NOTE: T currently holds  c_k*sy - spn + syf  = -(T_k). We work with
            # negT below: ksum = sum_k w_k * exp(negT - negTmax), loss_q = -negTmax - ln ksum.
            negT = T[:, : NP * K]

            # piece combine: PC_j = negT_j + m_j*(negT_{j+1} + m_{j+1}*negT_{j+2})
            # (supports groups spanning up to 3 pieces; host asserts that)
            pc1 = small.tile([PT, (NP + 1) * K], F32, tag="pc1")
            nc.vector.memset(pc1[:, NP * K :], 0.0)
            t2 = tmp.tile([PT, NP * K], F32, tag="t2")
            nc.vector.tensor_tensor(out=t2[:], in0=T[:, K :], in1=m5[:], op=MULT)
            nc.vector.tensor_tensor(out=pc1[:, : NP * K], in0=negT, in1=t2[:], op=ADD)
            pc = small.tile([PT, NP * K], F32, tag="pc")
            t3 = tmp.tile([PT, NP * K], F32, tag="t3")
            nc.vector.tensor_tensor(out=t3[:], in0=pc1[:, K :], in1=m5[:], op=MULT)
            nc.vector.tensor_tensor(out=pc[:], in0=negT, in1=t3[:], op=ADD)

            # stabilized LSE over k on combined pieces
            nmax = small.tile([PT, NP], F32, tag="nmax")
            pc3 = pc[:].rearrange("p (n k) -> p n k", k=K)
            nc.vector.tensor_reduce(out=nmax[:], in_=pc3, axis=AX, op=mybir.AluOpType.max)
            nmax_b = nmax[:].unsqueeze(2).broadcast_to((PT, NP, K))
            nc.vector.tensor_tensor(out=pc3, in0=pc3, in1=nmax_b, op=SUB)
            ex = small.tile([PT, NP * K], F32, tag="ex")
            nc.scalar.activation(out=ex[:], in_=pc[:], func=ACT.Exp)
            wm = small.tile([PT, NP * K], F32, tag="wm")
            nc.vector.tensor_tensor(out=wm[:], in0=ex[:], in1=wt[:], op=MULT)
            ks = small.tile([PT, NP], F32, tag="ks")
            nc.vector.tensor_reduce(
                out=ks[:], in_=wm[:].rearrange("p (n k) -> p n k", k=K), axis=AX, op=ADD
            )
            lk = small.tile([PT, NP], F32, tag="lk")
            nc.scalar.activation(out=lk[:], in_=ks[:], func=ACT.Ln)
            # loss_q = -log ksum_full = -(nmax + lk)  => accumulate (nmax+lk), negate at end
            dq = small.tile([PT, NP], F32, tag="dq")
            nc.vector.tensor_tensor(out=dq[:], in0=nmax[:], in1=lk[:], op=ADD)
            dqz = small.tile([PT, NP], F32, tag="dqz")
            nc.vector.tensor_tensor(out=dqz[:], in0=dq[:], in1=zt[:], op=MULT)
            if dbg:
                nc.sync.dma_start(out=dbg_t["dbg_T"][:], in_=negT[:])
                nc.sync.dma_start(out=dbg_t["dbg_P"][:], in_=pc[:])
                nc.sync.dma_start(out=dbg_t["dbg_dq"][:], in_=dqz[:])
            rs = small.tile([PT, 1], F32, tag="rs")
            nc.vector.tensor_reduce(out=rs[:], in_=dqz[:], axis=AX, op=ADD)
            negones = small.tile([PT, 1], F32, tag="negones")
            nc.vector.memset(negones[:], -1.0)
            tot_p = psum.tile([1, 1], F32)
            nc.tensor.matmul(out=tot_p[:], lhsT=rs[:], rhs=negones[:], start=True, stop=True)
            tot = small.tile([1, 1], F32, tag="tot")
            nc.vector.tensor_copy(out=tot[:], in_=tot_p[:])
            nc.sync.dma_start(out=loss_d[:], in_=tot[:])

    nc.compile()
    return nc


_NC_CACHE = {}


def get_nc(dbg=False):
    if dbg not in _NC_CACHE:
        _NC_CACHE[dbg] = build_nc(dbg)
    return _NC_CACHE[dbg]


def host_prep(y_true, y_pred, Z_idx, sig2b):
    """Sort by group; pack groups into fixed-width pieces across 1024 partitions."""
    y = np.asarray(y_true, dtype=np.float32).reshape(-1)
    f = np.asarray(y_pred, dtype=np.float32).reshape(-1)
    idx = np.asarray(Z_idx).astype(np.int32)
    n = y.shape[0]
    assert n == N

    perm = np.argsort(idx, kind="stable")
    sb = idx[perm]
    ys = y[perm]
    fs = f[perm]

    s = np.bincount(sb, minlength=Q).astype(np.int64)          # group sizes
    bin_start = np.concatenate(([0], np.cumsum(s)[:-1]))
    pcs = (s + WB - 1) // WB                                   # pieces per group
    piece_off = np.concatenate(([0], np.cumsum(pcs)[:-1]))
    total_pieces = int(pcs.sum())
    npt = -(-total_pieces // NPART)                            # target pieces/partition
    assert npt + int(pcs.max()) - 1 <= NP, (npt, int(pcs.max()))
    assert int(pcs.max()) <= 3, int(pcs.max())                 # combine depth

    nz = s > 0
    pid = np.zeros(Q, np.int64)
    pid[nz] = piece_off[nz] // npt                             # partition of each group
    assert pid.max() < NPART

    # local piece base per group: piece_off - first piece_off in its partition
    first_bin = np.searchsorted(pid[nz], np.arange(NPART), side="left")
    po_nz = piece_off[nz]
    part_first = np.zeros(NPART, np.int64)
    valid = first_bin < po_nz.shape[0]
    part_first[valid] = po_nz[np.minimum(first_bin, po_nz.shape[0] - 1)][valid]
    lpi = np.zeros(Q, np.int64)
    lpi[nz] = piece_off[nz] - part_first[pid[nz]]
    assert (lpi[nz] + pcs[nz]).max() <= NP

    # per-element slot
    b = sb.astype(np.int64)
    r = np.arange(n, dtype=np.int64) - bin_start[b]            # rank within group
    slot = pid[b] * FT + (lpi[b] + r // WB) * WB + (r % WB)

    Y = np.zeros(NPART * FT, np.float32)
    F = np.full(NPART * FT, FPAD, np.float32)
    Y[slot] = ys
    F[slot] = fs

    # masks
    mflat = np.zeros(NPART * NP, np.float32)
    for extra in (1, 2):
        sel = pcs > extra
        mflat[(pid[sel] * NP + lpi[sel] + (extra - 1)).astype(np.int64)] = 1.0
    zflat = np.zeros(NPART * NP, np.float32)
    zflat[(pid[nz] * NP + lpi[nz]).astype(np.int64)] = 1.0

    sig = float(np.asarray(sig2b).reshape(-1)[0])
    ck = (np.sqrt(2.0 * sig) * _XK).astype(np.float32)
    wk = (_WK / np.sqrt(np.pi)).astype(np.float32)
    cbias = np.tile(ck[None, :], (PT, 1)).astype(np.float32)
    escale = np.tile(np.exp(ck.astype(np.float64))[None, :], (PT, 1)).astype(np.float32)
    wtile = np.tile(wk[None, :], (PT, NP)).astype(np.float32)

    bf16 = mybir.dt.np(BF16)
    Y = Y.reshape(NPART, FT).astype(bf16)
    F = F.reshape(NPART, FT).astype(bf16)
    m5 = np.repeat(mflat.reshape(NPART, NP), K, axis=1)        # [NPART, NP*K]
    z2 = zflat.reshape(NPART, NP)

    in_maps = []
    for c in range(NCORES):
        sl = slice(c * PT, (c + 1) * PT)
        in_maps.append(
            {
                "ys": Y[sl],
                "fs": F[sl],
                "m5": m5[sl],
                "z": z2[sl],
                "cbias": cbias,
                "escale": escale,
                "wtile": wtile,
            }
        )
    return in_maps


def finish(results):
    total = sum(float(results[c]["loss"][0, 0]) for c in range(NCORES))
    return np.float32(total)


def kernel(y_true, y_pred, Z_idx, sig2b):
    nc = get_nc()
    in_maps = host_prep(y_true, y_pred, Z_idx, sig2b)
    res = run_bass_kernel_spmd(nc, in_maps, list(range(NCORES)))
    return finish(res.results)


# revision 31
# speedup vs baseline: 1.0286x; 1.0146x over previous
"""Trainium2 Bass kernel for nn_NLL_87333864997268 (GLMM logistic NLL with
Gauss-Hermite quadrature over a random intercept).

Math
----
With y in {0,1}, f the logit, c_k = sqrt(2*sig2b)*x_k (GH nodes):

    T[k,q] = sum_{i in group q} [ softplus(f_i + c_k) - y_i*(f_i + c_k) ]
           = SP_k[q] - YF[q] - c_k*SY[q]        (all three are segment sums)
    loss = -sum_q log( sum_k w_k/sqrt(pi) * exp(-T[k,q]) )

Strategy
--------
Host: stable sort by group id; pad each group to fixed-width W=52 "pieces"
(ceil(size/52) pieces per group, ~+37% slots); pack pieces into 1024
partition rows (8 cores x 128 partitions), never splitting a group across
partitions. All segment sums then become dense fixed-stride reductions on
device; host supplies two {0,1} masks: m (piece j continues into j+1) and
z (piece j is the first piece of its group).

Device (per core): 3 dense reduces for y and y*f, then per quadrature node
softplus (exp+ln, no Softplus table in this build) and a dense piece-sum
reduce; combine pieces via the m-mask (twice, supporting groups up to 3
pieces); stabilized log-sum-exp over the 5 nodes; z-masked total -> one
scalar per core. Host sums the 8 partials. Pad slots use f=-1e4, y=0 so
softplus(f+c)=0 exactly and they contribute nothing.
"""

import numpy as np

import concourse.bacc as bacc
import concourse.bass as bass
import concourse.mybir as mybir
import concourse.tile as tile
from concourse.bass_utils import run_bass_kernel_spmd

# problem constants (hardcoded per spec)
N = 4_194_304
Q = 100_000
NCORES = 8
K = 5            # Gauss-Hermite nodes
PT = 128         # partitions per core
NPART = NCORES * PT

WB = 52          # piece width (slots per piece)
NP = 108         # pieces per partition (static capacity)
FT = NP * WB     # slots per partition = 5616
FPAD = -10000.0  # pad logit: softplus(fpad+c)=0, y=0

_XK, _WK = np.polynomial.hermite.hermgauss(K)

F32 = mybir.dt.float32
BF16 = mybir.dt.bfloat16


def build_nc(dbg=False):
    """Build + compile the single-core SPMD Bass program."""
    nc = bacc.Bacc("TRN2", target_bir_lowering=False, debug=False)

    ys_d = nc.dram_tensor("ys", [PT, FT], BF16, kind="ExternalInput")
    fs_d = nc.dram_tensor("fs", [PT, FT], BF16, kind="ExternalInput")
    m5_d = nc.dram_tensor("m5", [PT, NP * K], F32, kind="ExternalInput")
    z_d = nc.dram_tensor("z", [PT, NP], F32, kind="ExternalInput")
    cbias_d = nc.dram_tensor("cbias", [PT, K], F32, kind="ExternalInput")
    escale_d = nc.dram_tensor("escale", [PT, K], F32, kind="ExternalInput")
    wtile_d = nc.dram_tensor("wtile", [PT, NP * K], F32, kind="ExternalInput")
    loss_d = nc.dram_tensor("loss", [1, 1], F32, kind="ExternalOutput")
    dbg_t = {}
    if dbg:
        dbg_t["dbg_T"] = nc.dram_tensor("dbg_T", [PT, NP * K], F32, kind="ExternalOutput")
        dbg_t["dbg_P"] = nc.dram_tensor("dbg_P", [PT, NP * K], F32, kind="ExternalOutput")
        dbg_t["dbg_dq"] = nc.dram_tensor("dbg_dq", [PT, NP], F32, kind="ExternalOutput")

    ADD = mybir.AluOpType.add
    SUB = mybir.AluOpType.subtract
    MULT = mybir.AluOpType.mult
    MIN = mybir.AluOpType.min
    ACT = mybir.ActivationFunctionType
    AX = mybir.AxisListType.X

    with tile.TileContext(nc) as tc:
        with (
            tc.tile_pool(name="big", bufs=1) as big,
            tc.tile_pool(name="tmp", bufs=4) as tmp,
            tc.tile_pool(name="small", bufs=1) as small,
            tc.tile_pool(name="psum", bufs=1, space="PSUM") as psum,
        ):
            yt = big.tile([PT, FT], BF16, tag="yt")
            ft = big.tile([PT, FT], BF16, tag="ft")

            cb = small.tile([PT, K], F32, tag="cb")
            es = small.tile([PT, K], F32, tag="es")
            m5 = small.tile([PT, NP * K], F32, tag="m5")
            zt = small.tile([PT, NP], F32, tag="zt")
            wt = small.tile([PT, NP * K], F32, tag="wt")
            nc.sync.dma_start(out=cb[:], in_=cbias_d[:])
            nc.sync.dma_start(out=es[:], in_=escale_d[:])

            # ---- big elementwise + piece-sum stage, in CH chunks for pipelining.
            # Piece sums via pairwise-halving tree (bf16 tensor_tensor hits the
            # DVE 2x packed mode; tensor_reduce never does) + final 13-reduce.
            CH = 2
            FC = FT // CH          # slots per chunk (2808)
            NC_ = NP // CH         # pieces per chunk (54)
            assert FC * CH == FT and NC_ * CH == NP

            sy = small.tile([PT, NP], BF16, tag="sy")
            syf = small.tile([PT, NP], BF16, tag="syf")
            # T has one extra zero piece so the combine can read a shifted view
            T = small.tile([PT, (NP + 1) * K], F32, tag="T")
            nc.vector.memset(T[:, NP * K :], 0.0)
            T3 = T[:, : NP * K].rearrange("p (n k) -> p n k", k=K)
            cvu = small.tile([PT, NP * K], F32, tag="cvu")
            cvu3 = cvu[:].rearrange("p (n k) -> p n k", k=K)

            def tree_sum(src_ap, out_ap, npieces, l1_engine=nc.vector, tags=""):
                """src [PT, npieces*WB] bf16 -> out [PT, npieces] bf16 piece sums."""
                s3 = src_ap.rearrange("p (n w) -> p n w", w=WB)
                h1 = tmp.tile([PT, npieces * 26], BF16, tag="h1" + tags)
                h13 = tmp.tile([PT, npieces * 13], BF16, tag="h13" + tags)
                h1v = h1[:].rearrange("p (n w) -> p n w", w=26)
                h13v = h13[:].rearrange("p (n w) -> p n w", w=13)
                l1_engine.tensor_tensor(out=h1v, in0=s3[:, :, 0:26], in1=s3[:, :, 26:52], op=ADD)
                nc.vector.tensor_tensor(out=h13v, in0=h1v[:, :, 0:13], in1=h1v[:, :, 13:26], op=ADD)
                nc.vector.tensor_reduce(out=out_ap, in_=h13v, axis=AX, op=ADD)

            with nc.allow_low_precision("piece sums are <=52 adds; bf16 keeps DVE 2x mode"):
                # phase 1: input DMAs + e^f per chunk (exp table loaded once)
                efs = []
                exp_insts = []
                for c in range(CH):
                    fsl = slice(c * FC, (c + 1) * FC)
                    nc.sync.dma_start(out=ft[:, fsl], in_=fs_d[:, fsl])
                    nc.sync.dma_start(out=yt[:, fsl], in_=ys_d[:, fsl])
                    ef = big.tile([PT, FC], BF16, tag=f"ef{c}")
                    exp_insts.append(
                        nc.scalar.activation(out=ef[:], in_=ft[:, fsl], func=ACT.Exp)
                    )
                    efs.append(ef)

                # phase 2: softplus(f+c_k) = ln(e^{c_k} * e^f + 1) + piece-sum trees
                # (emitted before the y-path so the in-order DVE queue isn't
                # blocked waiting on gpsimd's first levels)
                spns = {}
                from concourse.tile import add_dep_helper
                for c in range(CH):
                    for k in range(K):
                        sp = tmp.tile([PT, FC], BF16, tag="sp")
                        ln_inst = nc.scalar.activation(
                            out=sp[:], in_=efs[c][:], func=ACT.Ln, bias=1.0,
                            scale=es[:, k : k + 1],
                        )
                        if c == 0 and k == 0:
                            # keep both exp passes under one exp-table period:
                            # the first ln must not slot between them
                            add_dep_helper(
                                ln_inst.ins, exp_insts[-1].ins, sync=False,
                                reason="act table grouping",
                            )
                        spn = tmp.tile([PT, NC_], BF16, tag=f"spn{c}_{k}")
                        tree_sum(sp[:], spn[:], NC_)
                        spns[(c, k)] = spn

                # phase 3: y / y*f piece sums (first tree level on gpsimd) + T
                for c in range(CH):
                    fsl = slice(c * FC, (c + 1) * FC)
                    nsl = slice(c * NC_, (c + 1) * NC_)
                    yf = tmp.tile([PT, FC], BF16, tag="yf")
                    nc.gpsimd.tensor_tensor(out=yf[:], in0=yt[:, fsl], in1=ft[:, fsl], op=MULT)
                    tree_sum(yt[:, fsl], sy[:, nsl], NC_, l1_engine=nc.gpsimd, tags="y")
                    tree_sum(yf[:], syf[:, nsl], NC_, l1_engine=nc.gpsimd, tags="y")
                    for k in range(K):
                        nc.vector.scalar_tensor_tensor(
                            out=cvu3[:, nsl, k], in0=sy[:, nsl], scalar=cb[:, k : k + 1],
                            in1=syf[:, nsl], op0=MULT, op1=ADD,
                        )
                    for k in range(K):
                        # negated T pieces: T3_k = (c_k*sy + syf) - spn
                        nc.vector.tensor_tensor(
                            out=T3[:, nsl, k], in0=cvu3[:, nsl, k], in1=spns[(c, k)][:], op=SUB
                        )

            # mask/weight inputs are only needed at the tail
            nc.sync.dma_start(out=m5[:], in_=m5_d[:])
            nc.sync.dma_start(out=zt[:], in_=z_d[:])
            nc.sync.dma_start(out=wt[:], in_=wtile_d[:])
            # NOTE: T currently holds  c_k*sy - spn + syf  = -(T_k). We work with
            # negT below: ksum = sum_k w_k * exp(negT - negTmax), loss_q = -negTmax - ln ksum.
            negT = T[:, : NP * K]

            # piece combine: PC_j = negT_j + m_j*(negT_{j+1} + m_{j+1}*negT_{j+2})
            # (supports groups spanning up to 3 pieces; host asserts that)
            pc1 = small.tile([PT, (NP + 1) * K], F32, tag="pc1")
            nc.vector.memset(pc1[:, NP * K :], 0.0)
            t2 = tmp.tile([PT, NP * K], F32, tag="t2")
            nc.vector.tensor_tensor(out=t2[:], in0=T[:, K :], in1=m5[:], op=MULT)
            nc.vector.tensor_tensor(out=pc1[:, : NP * K], in0=negT, in1=t2[:], op=ADD)
            pc = small.tile([PT, NP * K], F32, tag="pc")
            t3 = tmp.tile([PT, NP * K], F32, tag="t3")
            nc.vector.tensor_tensor(out=t3[:], in0=pc1[:, K :], in1=m5[:], op=MULT)
            nc.vector.tensor_tensor(out=pc[:], in0=negT, in1=t3[:], op=ADD)

            # stabilized LSE over k on combined pieces
            nmax = small.tile([PT, NP], F32, tag="nmax")
            pc3 = pc[:].rearrange("p (n k) -> p n k", k=K)
            nc.vector.tensor_reduce(out=nmax[:], in_=pc3, axis=AX, op=mybir.AluOpType.max)
            nmax_b = nmax[:].unsqueeze(2).broadcast_to((PT, NP, K))
            nc.vector.tensor_tensor(out=pc3, in0=pc3, in1=nmax_b, op=SUB)
            ex = small.tile([PT, NP * K], F32, tag="ex")
            nc.scalar.activation(out=ex[:], in_=pc[:], func=ACT.Exp)
            wm = small.tile([PT, NP * K], F32, tag="wm")
            nc.vector.tensor_tensor(out=wm[:], in0=ex[:], in1=wt[:], op=MULT)
            ks = small.tile([PT, NP], F32, tag="ks")
            nc.vector.tensor_reduce(
                out=ks[:], in_=wm[:].rearrange("p (n k) -> p n k", k=K), axis=AX, op=ADD
            )
            lk = small.tile([PT, NP], F32, tag="lk")
            nc.scalar.activation(out=lk[:], in_=ks[:], func=ACT.Ln)
            # loss_q = -log ksum_full = -(nmax + lk)  => accumulate (nmax+lk), negate at end
            dq = small.tile([PT, NP], F32, tag="dq")
            nc.vector.tensor_tensor(out=dq[:], in0=nmax[:], in1=lk[:], op=ADD)
            dqz = small.tile([PT, NP], F32, tag="dqz")
            nc.vector.tensor_tensor(out=dqz[:], in0=dq[:], in1=zt[:], op=MULT)
            if dbg:
                nc.sync.dma_start(out=dbg_t["dbg_T"][:], in_=negT[:])
                nc.sync.dma_start(out=dbg_t["dbg_P"][:], in_=pc[:])
                nc.sync.dma_start(out=dbg_t["dbg_dq"][:], in_=dqz[:])
            rs = small.tile([PT, 1], F32, tag="rs")
            nc.vector.tensor_reduce(out=rs[:], in_=dqz[:], axis=AX, op=ADD)
            negones = small.tile([PT, 1], F32, tag="negones")
            nc.vector.memset(negones[:], -1.0)
            tot_p = psum.tile([1, 1], F32)
            nc.tensor.matmul(out=tot_p[:], lhsT=rs[:], rhs=negones[:], start=True, stop=True)
            tot = small.tile([1, 1], F32, tag="tot")
            nc.vector.tensor_copy(out=tot[:], in_=tot_p[:])
            nc.sync.dma_start(out=loss_d[:], in_=tot[:])

    nc.compile()
    return nc


_NC_CACHE = {}


def get_nc(dbg=False):
    if dbg not in _NC_CACHE:
        _NC_CACHE[dbg] = build_nc(dbg)
    return _NC_CACHE[dbg]


def host_prep(y_true, y_pred, Z_idx, sig2b):
    """Sort by group; pack groups into fixed-width pieces across 1024 partitions."""
    y = np.asarray(y_true, dtype=np.float32).reshape(-1)
    f = np.asarray(y_pred, dtype=np.float32).reshape(-1)
    idx = np.asarray(Z_idx).astype(np.int32)
    n = y.shape[0]
    assert n == N

    perm = np.argsort(idx, kind="stable")
    sb = idx[perm]
    ys = y[perm]
    fs = f[perm]

    s = np.bincount(sb, minlength=Q).astype(np.int64)          # group sizes
    bin_start = np.concatenate(([0], np.cumsum(s)[:-1]))
    pcs = (s + WB - 1) // WB                                   # pieces per group
    piece_off = np.concatenate(([0], np.cumsum(pcs)[:-1]))
    total_pieces = int(pcs.sum())
    npt = -(-total_pieces // NPART)                            # target pieces/partition
    assert npt + int(pcs.max()) - 1 <= NP, (npt, int(pcs.max()))
    assert int(pcs.max()) <= 3, int(pcs.max())                 # combine depth

    nz = s > 0
    pid = np.zeros(Q, np.int64)
    pid[nz] = piece_off[nz] // npt                             # partition of each group
    assert pid.max() < NPART

    # local piece base per group: piece_off - first piece_off in its partition
    first_bin = np.searchsorted(pid[nz], np.arange(NPART), side="left")
    po_nz = piece_off[nz]
    part_first = np.zeros(NPART, np.int64)
    valid = first_bin < po_nz.shape[0]
    part_first[valid] = po_nz[np.minimum(first_bin, po_nz.shape[0] - 1)][valid]
    lpi = np.zeros(Q, np.int64)
    lpi[nz] = piece_off[nz] - part_first[pid[nz]]
    assert (lpi[nz] + pcs[nz]).max() <= NP

    # per-element slot
    b = sb.astype(np.int64)
    r = np.arange(n, dtype=np.int64) - bin_start[b]            # rank within group
    slot = pid[b] * FT + (lpi[b] + r // WB) * WB + (r % WB)

    Y = np.zeros(NPART * FT, np.float32)
    F = np.full(NPART * FT, FPAD, np.float32)
    Y[slot] = ys
    F[slot] = fs

    # masks
    mflat = np.zeros(NPART * NP, np.float32)
    for extra in (1, 2):
        sel = pcs > extra
        mflat[(pid[sel] * NP + lpi[sel] + (extra - 1)).astype(np.int64)] = 1.0
    zflat = np.zeros(NPART * NP, np.float32)
    zflat[(pid[nz] * NP + lpi[nz]).astype(np.int64)] = 1.0

    sig = float(np.asarray(sig2b).reshape(-1)[0])
    ck = (np.sqrt(2.0 * sig) * _XK).astype(np.float32)
    wk = (_WK / np.sqrt(np.pi)).astype(np.float32)
    cbias = np.tile(ck[None, :], (PT, 1)).astype(np.float32)
    escale = np.tile(np.exp(ck.astype(np.float64))[None, :], (PT, 1)).astype(np.float32)
    wtile = np.tile(wk[None, :], (PT, NP)).astype(np.float32)

    bf16 = mybir.dt.np(BF16)
    Y = Y.reshape(NPART, FT).astype(bf16)
    F = F.reshape(NPART, FT).astype(bf16)
    m5 = np.repeat(mflat.reshape(NPART, NP), K, axis=1)        # [NPART, NP*K]
    z2 = zflat.reshape(NPART, NP)

    in_maps = []
    for c in range(NCORES):
        sl = slice(c * PT, (c + 1) * PT)
        in_maps.append(
            {
                "ys": Y[sl],
                "fs": F[sl],
                "m5": m5[sl],
                "z": z2[sl],
                "cbias": cbias,
                "escale": escale,
                "wtile": wtile,
            }
        )
    return in_maps


def finish(results):
    total = sum(float(results[c]["loss"][0, 0]) for c in range(NCORES))
    return np.float32(total)


def kernel(y_true, y_pred, Z_idx, sig2b):
    nc = get_nc()
    in_maps = host_prep(y_true, y_pred, Z_idx, sig2b)
    res = run_bass_kernel_spmd(nc, in_maps, list(range(NCORES)))
    return finish(res.results)


# revision 34
# speedup vs baseline: 1.0324x; 1.0038x over previous
"""Trainium2 Bass kernel for nn_NLL_87333864997268 (GLMM logistic NLL with
Gauss-Hermite quadrature over a random intercept).

Math
----
With y in {0,1}, f the logit, c_k = sqrt(2*sig2b)*x_k (GH nodes):

    T[k,q] = sum_{i in group q} [ softplus(f_i + c_k) - y_i*(f_i + c_k) ]
           = SP_k[q] - YF[q] - c_k*SY[q]        (all three are segment sums)
    loss = -sum_q log( sum_k w_k/sqrt(pi) * exp(-T[k,q]) )

Strategy
--------
Host: stable sort by group id; pad each group to fixed-width W=52 "pieces"
(ceil(size/52) pieces each); pack pieces into 2048 half-partition rows
(8 cores x 128 partitions x 2 chunks), never splitting a group across a
half-partition. All segment sums then become dense fixed-stride reductions;
host supplies {0,1} masks: m (piece j continues into piece j+1) and z
(piece j is the first piece of its group). Pad slots use f=-1e4, y=0 so
softplus(f+c)=0 exactly and they contribute nothing.

Device (per core, two half-size chunks pipelined): softplus via one Exp pass
and per-node Ln(e^{c_k}*e^f + 1) (shared e^f; exp/ln live in different ACT
table sets, so the two Exp passes are grouped); piece sums via a pairwise
bf16 halving tree (hits the DVE 2x packed mode; tensor_reduce does not) plus
a final 13-wide reduce; y/y*f trees on the otherwise-idle GPSIMD; per-chunk
piece-combine + stabilized log-sum-exp so chunk 0's tail overlaps chunk 1's
compute; single scalar out per core, host sums the 8 partials.
"""

import numpy as np

import concourse.bacc as bacc
import concourse.bass as bass
import concourse.mybir as mybir
import concourse.tile as tile
from concourse.bass_utils import run_bass_kernel_spmd
from concourse.tile import add_dep_helper

# problem constants (hardcoded per spec)
N = 4_194_304
Q = 100_000
NCORES = 8
K = 5            # Gauss-Hermite nodes
PT = 128         # partitions per core

WB = 52          # piece width (slots per piece)
CH = 2           # chunks (half-partitions) -- groups never cross a chunk
NPH = 55         # pieces per half-partition (static capacity)
NP = NPH * CH    # pieces per partition = 110
FT = NP * WB     # slots per partition = 5720
FC = FT // CH    # slots per chunk = 2860
VPART = NCORES * PT * CH  # 2048 packing rows
FPAD = -10000.0  # pad logit: softplus(fpad+c)=0, y=0

_XK, _WK = np.polynomial.hermite.hermgauss(K)

F32 = mybir.dt.float32
BF16 = mybir.dt.bfloat16


def build_nc(dbg=False):
    """Build + compile the single-core SPMD Bass program."""
    nc = bacc.Bacc("TRN2", target_bir_lowering=False, debug=False)

    ys_d = nc.dram_tensor("ys", [PT, FT], BF16, kind="ExternalInput")
    fs_d = nc.dram_tensor("fs", [PT, FT], BF16, kind="ExternalInput")
    m5_d = nc.dram_tensor("m5", [PT, NP * K], BF16, kind="ExternalInput")
    z_d = nc.dram_tensor("z", [PT, NP], F32, kind="ExternalInput")
    cbias_d = nc.dram_tensor("cbias", [PT, K], F32, kind="ExternalInput")
    escale_d = nc.dram_tensor("escale", [PT, K], F32, kind="ExternalInput")
    wtile_d = nc.dram_tensor("wtile", [PT, NP * K], F32, kind="ExternalInput")
    loss_d = nc.dram_tensor("loss", [1, 1], F32, kind="ExternalOutput")

    ADD = mybir.AluOpType.add
    SUB = mybir.AluOpType.subtract
    MULT = mybir.AluOpType.mult
    MAX = mybir.AluOpType.max
    ACT = mybir.ActivationFunctionType
    AX = mybir.AxisListType.X

    with tile.TileContext(nc) as tc:
        with (
            tc.tile_pool(name="big", bufs=1) as big,
            tc.tile_pool(name="tmp", bufs=4) as tmp,
            tc.tile_pool(name="small", bufs=1) as small,
            tc.tile_pool(name="psum", bufs=1, space="PSUM") as psum,
        ):
            yt = big.tile([PT, FT], BF16, tag="yt")
            ft = big.tile([PT, FT], BF16, tag="ft")

            cb = small.tile([PT, K], F32, tag="cb")
            es = small.tile([PT, K], F32, tag="es")
            m5 = small.tile([PT, NP * K], BF16, tag="m5")
            zt = small.tile([PT, NP], F32, tag="zt")
            wt = small.tile([PT, NP * K], F32, tag="wt")

            sy = small.tile([PT, NP], BF16, tag="sy")
            syf = small.tile([PT, NP], BF16, tag="syf")
            # negated-T pieces, bf16, with one extra zero piece per chunk so the
            # combine can read a shifted view ([NPH+1 pieces] * K per chunk)
            T = small.tile([PT, CH * (NPH + 1) * K], BF16, tag="T")
            cvu = small.tile([PT, NP * K], BF16, tag="cvu")
            dqz = small.tile([PT, NP], F32, tag="dqz")

            def tsl(c):
                """chunk-c T view [PT, (NPH+1)*K], trailing K entries are zero."""
                return T[:, c * (NPH + 1) * K : (c + 1) * (NPH + 1) * K]

            for c in range(CH):
                nc.vector.memset(tsl(c)[:, NPH * K :], 0.0)

            def tree_sum(src_ap, out_ap, npieces, l1_engine=nc.vector, tags=""):
                """src [PT, npieces*WB] bf16 -> out [PT, npieces] bf16 piece sums."""
                s3 = src_ap.rearrange("p (n w) -> p n w", w=WB)
                h1 = tmp.tile([PT, npieces * 26], BF16, tag="h1" + tags)
                h13 = tmp.tile([PT, npieces * 13], BF16, tag="h13" + tags)
                h1v = h1[:].rearrange("p (n w) -> p n w", w=26)
                h13v = h13[:].rearrange("p (n w) -> p n w", w=13)
                l1_engine.tensor_tensor(out=h1v, in0=s3[:, :, 0:26], in1=s3[:, :, 26:52], op=ADD)
                nc.vector.tensor_tensor(out=h13v, in0=h1v[:, :, 0:13], in1=h1v[:, :, 13:26], op=ADD)
                nc.vector.tensor_reduce(out=out_ap, in_=h13v, axis=AX, op=ADD)

            with nc.allow_low_precision("piece sums are <=52 adds; bf16 keeps DVE 2x mode"):
                # phase 1: input DMAs + e^f per chunk (exp table loaded once)
                efs = []
                exp_insts = []
                for c in range(CH):
                    fsl = slice(c * FC, (c + 1) * FC)
                    nc.sync.dma_start(out=ft[:, fsl], in_=fs_d[:, fsl])
                    nc.sync.dma_start(out=yt[:, fsl], in_=ys_d[:, fsl])
                    if c == 0:
                        nc.sync.dma_start(out=cb[:], in_=cbias_d[:])
                        nc.sync.dma_start(out=es[:], in_=escale_d[:])
                    ef = big.tile([PT, FC], BF16, tag=f"ef{c}")
                    exp_insts.append(
                        nc.scalar.activation(out=ef[:], in_=ft[:, fsl], func=ACT.Exp)
                    )
                    efs.append(ef)

                # mask/weight inputs ride behind the element data on the queue
                nc.sync.dma_start(out=m5[:], in_=m5_d[:])
                nc.sync.dma_start(out=zt[:], in_=z_d[:])
                nc.sync.dma_start(out=wt[:], in_=wtile_d[:])

                # phase 2+3 per chunk: softplus piece sums, y-path, T assembly,
                # then per-chunk combine + LSE (chunk 0's tail overlaps chunk 1)
                for c in range(CH):
                    fsl = slice(c * FC, (c + 1) * FC)
                    nsl = slice(c * NPH, (c + 1) * NPH)
                    Tc = tsl(c)
                    T3 = Tc[:, : NPH * K].rearrange("p (n k) -> p n k", k=K)
                    cvu3 = cvu[:, c * NPH * K : (c + 1) * NPH * K].rearrange(
                        "p (n k) -> p n k", k=K
                    )
                    m5c = m5[:, c * NPH * K : (c + 1) * NPH * K]

                    # y / y*f piece sums: first tree level on gpsimd
                    yf = tmp.tile([PT, FC], BF16, tag="yf")
                    nc.gpsimd.tensor_tensor(out=yf[:], in0=yt[:, fsl], in1=ft[:, fsl], op=MULT)
                    tree_sum(yt[:, fsl], sy[:, nsl], NPH, l1_engine=nc.gpsimd, tags="y")
                    tree_sum(yf[:], syf[:, nsl], NPH, l1_engine=nc.gpsimd, tags="y")
                    for k in range(K):
                        nc.vector.scalar_tensor_tensor(
                            out=cvu3[:, :, k], in0=sy[:, nsl], scalar=cb[:, k : k + 1],
                            in1=syf[:, nsl], op0=MULT, op1=ADD,
                        )

                    # softplus(f+c_k) = ln(e^{c_k} * e^f + 1), piece-sum trees
                    for k in range(K):
                        sp = tmp.tile([PT, FC], BF16, tag="sp")
                        ln_inst = nc.scalar.activation(
                            out=sp[:], in_=efs[c][:], func=ACT.Ln, bias=1.0,
                            scale=es[:, k : k + 1],
                        )
                        if c == 0 and k == 0:
                            # keep both exp passes inside one exp-table period
                            add_dep_helper(
                                ln_inst.ins, exp_insts[-1].ins, sync=False,
                                reason="act table grouping",
                            )
                        spn = tmp.tile([PT, NPH], BF16, tag="spn")
                        tree_sum(sp[:], spn[:], NPH)
                        # negated T pieces: T3_k = (c_k*sy + syf) - spn
                        nc.vector.tensor_tensor(
                            out=T3[:, :, k], in0=cvu3[:, :, k], in1=spn[:], op=SUB
                        )

                    # piece combine: PC_j = T_j + m_j*(T_{j+1} + m_{j+1}*T_{j+2})
                    negT = Tc[:, : NPH * K]
                    pc1 = tmp.tile([PT, (NPH + 1) * K], BF16, tag="pc1")
                    nc.vector.memset(pc1[:, NPH * K :], 0.0)
                    t2 = tmp.tile([PT, NPH * K], BF16, tag="t2")
                    nc.vector.tensor_tensor(out=t2[:], in0=Tc[:, K:], in1=m5c, op=MULT)
                    nc.vector.tensor_tensor(out=pc1[:, : NPH * K], in0=negT, in1=t2[:], op=ADD)
                    pcc = tmp.tile([PT, NPH * K], BF16, tag="pcc")
                    t3 = tmp.tile([PT, NPH * K], BF16, tag="t3")
                    nc.vector.tensor_tensor(out=t3[:], in0=pc1[:, K:], in1=m5c, op=MULT)
                    nc.vector.tensor_tensor(out=pcc[:], in0=negT, in1=t3[:], op=ADD)

                    # stabilized LSE over k
                    nmax = tmp.tile([PT, NPH], BF16, tag="nmax")
                    pc3 = pcc[:].rearrange("p (n k) -> p n k", k=K)
                    nc.vector.tensor_reduce(out=nmax[:], in_=pc3, axis=AX, op=MAX)
                    nmax_b = nmax[:].unsqueeze(2).broadcast_to((PT, NPH, K))
                    nc.vector.tensor_tensor(out=pc3, in0=pc3, in1=nmax_b, op=SUB)
                    ex = tmp.tile([PT, NPH * K], BF16, tag="ex")
                    nc.scalar.activation(out=ex[:], in_=pcc[:], func=ACT.Exp)
                    wm = tmp.tile([PT, NPH * K], F32, tag="wm")
                    nc.vector.tensor_tensor(
                        out=wm[:], in0=ex[:],
                        in1=wt[:, c * NPH * K : (c + 1) * NPH * K], op=MULT,
                    )
                    ks = tmp.tile([PT, NPH], F32, tag="ks")
                    nc.vector.tensor_reduce(
                        out=ks[:], in_=wm[:].rearrange("p (n k) -> p n k", k=K),
                        axis=AX, op=ADD,
                    )
                    lk = tmp.tile([PT, NPH], F32, tag="lk")
                    nc.scalar.activation(out=lk[:], in_=ks[:], func=ACT.Ln)
                    # loss_q = -(nmax + lk); z-masked; negation folded into the
                    # final matmul's -1 column
                    dq = tmp.tile([PT, NPH], F32, tag="dq")
                    nc.vector.tensor_tensor(out=dq[:], in0=nmax[:], in1=lk[:], op=ADD)
                    nc.vector.tensor_tensor(
                        out=dqz[:, nsl], in0=dq[:], in1=zt[:, nsl], op=MULT
                    )

            rs = small.tile([PT, 1], F32, tag="rs")
            nc.vector.tensor_reduce(out=rs[:], in_=dqz[:], axis=AX, op=ADD)
            negones = small.tile([PT, 1], F32, tag="negones")
            nc.vector.memset(negones[:], -1.0)
            tot_p = psum.tile([1, 1], F32)
            nc.tensor.matmul(out=tot_p[:], lhsT=rs[:], rhs=negones[:], start=True, stop=True)
            tot = small.tile([1, 1], F32, tag="tot")
            nc.vector.tensor_copy(out=tot[:], in_=tot_p[:])
            nc.sync.dma_start(out=loss_d[:], in_=tot[:])

    nc.compile()
    return nc


_NC_CACHE = {}


def get_nc(dbg=False):
    if dbg not in _NC_CACHE:
        _NC_CACHE[dbg] = build_nc(dbg)
    return _NC_CACHE[dbg]


def host_prep(y_true, y_pred, Z_idx, sig2b):
    """Sort by group; pack groups into fixed-width pieces across 2048 rows."""
    y = np.asarray(y_true, dtype=np.float32).reshape(-1)
    f = np.asarray(y_pred, dtype=np.float32).reshape(-1)
    idx = np.asarray(Z_idx).astype(np.int32)
    n = y.shape[0]
    assert n == N

    perm = np.argsort(idx, kind="stable")
    sb = idx[perm]
    ys = y[perm]
    fs = f[perm]

    s = np.bincount(sb, minlength=Q).astype(np.int64)          # group sizes
    bin_start = np.concatenate(([0], np.cumsum(s)[:-1]))
    pcs = (s + WB - 1) // WB                                   # pieces per group
    piece_off = np.concatenate(([0], np.cumsum(pcs)[:-1]))
    total_pieces = int(pcs.sum())
    npt = -(-total_pieces // VPART)                            # target pieces/row
    assert npt + int(pcs.max()) - 1 <= NPH, (npt, int(pcs.max()))
    assert int(pcs.max()) <= 3, int(pcs.max())                 # combine depth

    nz = s > 0
    pid = np.zeros(Q, np.int64)
    pid[nz] = piece_off[nz] // npt                             # packing row of group
    assert pid.max() < VPART

    # local piece base per group within its packing row
    first_bin = np.searchsorted(pid[nz], np.arange(VPART), side="left")
    po_nz = piece_off[nz]
    part_first = np.zeros(VPART, np.int64)
    valid = first_bin < po_nz.shape[0]
    part_first[valid] = po_nz[np.minimum(first_bin, po_nz.shape[0] - 1)][valid]
    lpi = np.zeros(Q, np.int64)
    lpi[nz] = piece_off[nz] - part_first[pid[nz]]
    assert (lpi[nz] + pcs[nz]).max() <= NPH

    # packing row v -> (partition p, chunk c): v = p*CH + c  (chunks are the
    # two halves of a partition's piece range)
    # per-element slot (within the global [NCORES*PT, FT] layout)
    b = sb.astype(np.int64)
    r = np.arange(n, dtype=np.int64) - bin_start[b]            # rank within group
    v = pid[b]
    p_glob = v // CH
    cch = v % CH
    slot = p_glob * FT + (cch * NPH + lpi[b] + r // WB) * WB + (r % WB)

    NPART = NCORES * PT
    Y = np.zeros(NPART * FT, np.float32)
    F = np.full(NPART * FT, FPAD, np.float32)
    Y[slot] = ys
    F[slot] = fs

    # masks over pieces: layout [NPART, NP] with piece index = c*NPH + lpi
    mflat = np.zeros(NPART * NP, np.float32)
    vz = pid[nz]
    base = (vz // CH) * NP + (vz % CH) * NPH + lpi[nz]
    for extra in (1, 2):
        sel = pcs[nz] > extra
        mflat[(base[sel] + (extra - 1)).astype(np.int64)] = 1.0
    zflat = np.zeros(NPART * NP, np.float32)
    zflat[base.astype(np.int64)] = 1.0

    sig = float(np.asarray(sig2b).reshape(-1)[0])
    ck = (np.sqrt(2.0 * sig) * _XK).astype(np.float32)
    wk = (_WK / np.sqrt(np.pi)).astype(np.float32)
    cbias = np.tile(ck[None, :], (PT, 1)).astype(np.float32)
    escale = np.tile(np.exp(ck.astype(np.float64))[None, :], (PT, 1)).astype(np.float32)
    wtile = np.tile(wk[None, :], (PT, NP)).astype(np.float32)

    bf16 = mybir.dt.np(BF16)
    Y = Y.reshape(NPART, FT).astype(bf16)
    F = F.reshape(NPART, FT).astype(bf16)
    m5 = np.repeat(mflat.reshape(NPART, NP), K, axis=1).astype(bf16)
    z2 = zflat.reshape(NPART, NP)

    in_maps = []
    for c in range(NCORES):
        sl = slice(c * PT, (c + 1) * PT)
        in_maps.append(
            {
                "ys": Y[sl],
                "fs": F[sl],
                "m5": m5[sl],
                "z": z2[sl],
                "cbias": cbias,
                "escale": escale,
                "wtile": wtile,
            }
        )
    return in_maps


def finish(results):
    total = sum(float(results[c]["loss"][0, 0]) for c in range(NCORES))
    return np.float32(total)


def kernel(y_true, y_pred, Z_idx, sig2b):
    nc = get_nc()
    in_maps = host_prep(y_true, y_pred, Z_idx, sig2b)
    res = run_bass_kernel_spmd(nc, in_maps, list(range(NCORES)))
    return finish(res.results)
